# revision 26
# baseline (speedup 1.0000x reference)
"""Multi-head attention layer on 8 Trainium2 NeuronCores.

Sharding: data-parallel over the batch (4) x tensor-parallel over head
groups (2 groups of 8 heads) -> 8 independent shards, one per core. Scores
and softmax stay device-local per head; no collectives are needed. Each
core computes, for its (batch, 8-head) shard:

    q/k projections in transposed layout  qT/kT = W.T @ x.T   [512, 2048]
    v projection in natural layout        v = x @ Wv          [2048, 512]
    per head: sT = k qT (scores transposed, [sk, sq]); e = exp(sT/8 + mask)
    ctxT' = [v | 1].T @ e  -> rows 0..63 = unnormalized ctx.T, row 64 = sum(e)
    transpose ctxT' back, divide by the sum row -> ctx [2048, 512]

Computing scores transposed makes the softmax reduction the contraction
dim of the PV matmul (ones-column trick), so no [S,S] transposes are ever
needed. Matmul operands are bf16 (cast for free by SWDGE DMA), accumulation
is fp32 in PSUM; exp runs on the scalar engine reading PSUM directly.
"""

import sys

if "/opt/trn_rl_repo" not in sys.path:
    sys.path.insert(0, "/opt/trn_rl_repo")

import numpy as np

import concourse.bass as bass
import concourse.mybir as mybir
import concourse.tile as tile
from concourse import bacc
from concourse.bass import ds, ts
from concourse.bass_utils import run_bass_kernel_spmd
from concourse.masks import make_identity
from concourse.tile import add_dep_helper

F32 = mybir.dt.float32
F32R = mybir.dt.float32r
BF16 = mybir.dt.bfloat16
I32 = mybir.dt.int32
EXPF = mybir.ActivationFunctionType.Exp
COPYF = mybir.ActivationFunctionType.Copy

B, S, D = 4, 2048, 1024
H, DH = 16, 64
NEG_INF = -10000.0
N_CORES = 8
HL = H // 2          # heads per core (8)
DHL = HL * DH        # local qkv width (512)
KD = D // 128        # contraction slices (8)
ST = S // 128        # sequence tiles (16)
SC = S // 512        # sequence 512-chunks (4)
SCALE = 1.0 / 8.0    # 1/sqrt(DH)
EXP_G = 2            # sk-tiles per exp instruction (2 f32 PSUM banks)


def _build(use_mask: bool, use_bv: bool):
    nc = bacc.Bacc("TRN2", target_bir_lowering=False, debug=True)

    x_d = nc.dram_tensor("x", [S, D], F32, kind="ExternalInput")
    wq_d = nc.dram_tensor("wq", [D, DHL], F32, kind="ExternalInput")
    wk_d = nc.dram_tensor("wk", [D, DHL], F32, kind="ExternalInput")
    wv_d = nc.dram_tensor("wv", [D, DHL], F32, kind="ExternalInput")
    bq_d = nc.dram_tensor("bq", [DHL], F32, kind="ExternalInput")
    bk_d = nc.dram_tensor("bk", [DHL], F32, kind="ExternalInput")
    bv_d = nc.dram_tensor("bv", [DHL], F32, kind="ExternalInput")
    mask_d = nc.dram_tensor("mask", [S], I32, kind="ExternalInput")
    out_d = nc.dram_tensor("out", [S, DHL], F32, kind="ExternalOutput")

    with tile.TileContext(nc) as tc:
        with (
            tc.tile_pool(name="persist", bufs=1) as persist,
            tc.tile_pool(name="xload", bufs=3) as xload,
            tc.tile_pool(name="qkv", bufs=1) as qkv,
        ):
            # touch Exp first so the ACT table set (which also covers the
            # Copy casts below) loads during the intro DMA window instead of
            # stalling the first real exp in the attention pipeline
            warm = persist.tile([128, 1], F32, name="warm")
            nc.gpsimd.memset(warm[:], 0.0)
            nc.scalar.activation(warm[:], warm[:], EXPF)

            ident_bf = persist.tile([128, 128], BF16, name="ident_bf")
            make_identity(nc, ident_bf[:])
            ident_f = persist.tile([128, 128], F32, name="ident_f")
            make_identity(nc, ident_f[:])

            # biases as [128, 4] columns (partition = dh within 128-block)
            bqc = persist.tile([128, 4], F32, name="bqc")
            bkc = persist.tile([128, 4], F32, name="bkc")
            nc.sync.dma_start(bqc[:], bq_d.rearrange("(m p) -> p m", p=128))
            nc.sync.dma_start(bkc[:], bk_d.rearrange("(m p) -> p m", p=128))

            if use_mask:
                mski = persist.tile([128, ST], I32, name="mski")
                nc.sync.dma_start(mski[:], mask_d.rearrange("(t p) -> p t", p=128))
                adder = persist.tile([128, ST], F32, name="adder")
                # (1 - m) * NEG_INF == m * (-NEG_INF) + NEG_INF
                nc.scalar.activation(adder[:], mski[:], COPYF, bias=-NEG_INF * 1.0,
                                     scale=float(NEG_INF) * -1.0)
                # note: Copy computes in*scale + bias with float bias
                # in*(-NEG_INF) + NEG_INF: m=1 -> 0, m=0 -> NEG_INF

            # weights, cast to bf16 by SWDGE during the DMA (issued after the
            # x loads below -- x is on the critical path, W is not needed
            # until the first q/k projection ~45us in)
            w_all = persist.tile([128, 3, KD, DHL], BF16, name="w_all")

            if use_bv:
                bv_bf = persist.tile([1, DHL], BF16, name="bv_bf")
                nc.gpsimd.dma_start(bv_bf[:], bv_d[None, :])
                ones_col = persist.tile([1, 128], BF16, name="ones_col")
                nc.gpsimd.memset(ones_col[:], 1.0)

            qT = qkv.tile([128, 4, S], BF16, name="qT")
            kT = qkv.tile([128, 4, S], BF16, name="kT")
            vv = qkv.tile([128, ST, HL, DH + 1], BF16, name="vv")
            nc.gpsimd.memset(vv[:, :, :, DH:DH + 1], 1.0)

            # ---------------- phases: x.T -> qkv(jit) -> attention ------
            with (
                tc.tile_pool(name="xt", bufs=1) as xtp,
                tc.tile_pool(name="expp", bufs=2) as expp,
                tc.tile_pool(name="outp", bufs=2) as outp,
                tc.tile_pool(name="ctxs", bufs=4) as ctxs,
                tc.tile_pool(name="psA", bufs=2, space="PSUM") as ps_qk,
                tc.tile_pool(name="pssc", bufs=2, space="PSUM") as ps_sc,
                tc.tile_pool(name="psctx", bufs=2, space="PSUM") as ps_ctx,
            ):
                xT = xtp.tile([128, KD, S], BF16, name="xT")
                for st in range(ST):
                    x_f = xload.tile([128, D], F32, name="x_f")
                    x_dma = nc.sync.dma_start(x_f[:], x_d[ts(st, 128), :])
                    x_bf = xload.tile([128, D], BF16, name="x_bf")
                    nc.scalar.activation(x_bf[:], x_f[:], COPYF)
                    if st == 3:
                        # weight DMAs gated behind the first x tiles: both
                        # queues share the SDMA engines/HBM, and x is the
                        # critical path (W isn't consumed until ~15us in)
                        for i, wd in enumerate((wq_d, wk_d, wv_d)):
                            w_dma = nc.gpsimd.dma_start(
                                w_all[:, i, :, :],
                                wd.rearrange("(k p) m -> p k m", p=128),
                            )
                            add_dep_helper(
                                w_dma.ins, x_dma.ins,
                                reason="W cast-DMA after first x tiles",
                            )
                    for half in range(2):
                        ps_t = ps_ctx.tile([128, 4, 128], BF16, name="ctx")
                        for j in range(4):
                            dt_ = half * 4 + j
                            nc.tensor.transpose(
                                ps_t[:, j, :], x_bf[:, ts(dt_, 128)], ident_bf[:]
                            )
                        nc.vector.tensor_copy(
                            xT[:, ds(half * 4, 4), ts(st, 128)], ps_t[:]
                        )

                # q.T / k.T for one head-pair block; n4 in pairs so only two
                # PSUM banks are needed (weights reused across the pair)
                def emit_qk_gen(mb):
                    for proj, (dst, bias_c) in enumerate(((qT, bqc), (kT, bkc))):
                        for pair in range(2):
                            pss = [
                                ps_qk.tile([128, 512], F32, name="qk")
                                for _ in range(2)
                            ]
                            for kd in range(KD):
                                lhs = w_all[:, proj, kd, ts(mb, 128)]
                                for u in range(2):
                                    n4 = pair * 2 + u
                                    nc.tensor.matmul(
                                        pss[u][:], lhs, xT[:, kd, ts(n4, 512)],
                                        start=(kd == 0), stop=(kd == KD - 1),
                                    )
                                yield
                            for u in range(2):
                                nc.vector.tensor_scalar_add(
                                    dst[:, mb, ts(pair * 2 + u, 512)], pss[u][:],
                                    bias_c[:, mb:mb + 1],
                                )
                            yield

                def emit_qk(mb):
                    for _ in emit_qk_gen(mb):
                        pass

                emit_qk(0)

                # v in natural layout [S, dh_local] (+1.0 ones column)
                for st in range(ST):
                    psv = ps_qk.tile([128, 512], F32, name="qk")
                    for kd in range(KD):
                        nc.tensor.matmul(
                            psv[:], xT[:, kd, ts(st, 128)], w_all[:, 2, kd, :],
                            start=(kd == 0), stop=(kd == KD - 1),
                        )
                    nc.vector.tensor_copy(vv[:, st, :, 0:DH], psv[:])

                if use_bv:
                    bvp = ps_qk.tile([128, DHL], F32, name="qk")
                    nc.tensor.matmul(bvp[:], ones_col[:], bv_bf[:],
                                     start=True, stop=True)
                    bv_bc = persist.tile([128, DHL], F32, name="bv_bc")
                    nc.vector.tensor_copy(bv_bc[:], bvp[:])

                # ---------------- attention, head-pair outer ---------------
                NG = ST // EXP_G
                for hp in range(4):
                    if hp > 0:
                        emit_qk(hp)
                    heads = (2 * hp, 2 * hp + 1)
                    rows = (slice(0, 64), slice(64, 128))
                    for c in range(SC):
                        out_sb = outp.tile([128, 4, 128], F32, name="out_sb")
                        exps = [
                            expp.tile([128, ST, 512], BF16, name=f"exp{i}")
                            for i in range(2)
                        ]
                        ctxp = [
                            ps_ctx.tile([DH + 1, 512], F32, name="ctx")
                            for i in range(2)
                        ]

                        def emit_scores(g):
                            pair = []
                            for i in range(2):
                                t = ps_sc.tile([128, EXP_G, 512], F32, name="sc")
                                pair.append(t)
                                for jj in range(EXP_G):
                                    j = g * EXP_G + jj
                                    nc.tensor.matmul(
                                        t[:, jj, :],
                                        kT[rows[i], hp, ts(j, 128)],
                                        qT[rows[i], hp, ts(c, 512)],
                                        start=True, stop=True,
                                        tile_position=(64 * i, 0),
                                    )
                            return pair

                        pending = emit_scores(0)
                        for g in range(NG):
                            cur = pending
                            pending = emit_scores(g + 1) if g + 1 < NG else None
                            for i in range(2):
                                if use_mask:
                                    for jj in range(EXP_G):
                                        j = g * EXP_G + jj
                                        nc.scalar.activation(
                                            exps[i][:, j, :], cur[i][:, jj, :],
                                            EXPF, bias=adder[:, j:j + 1],
                                            scale=SCALE,
                                        )
                                else:
                                    nc.scalar.activation(
                                        exps[i][:, ds(g * EXP_G, EXP_G), :],
                                        cur[i][:], EXPF, scale=SCALE,
                                    )
                            for jj in range(EXP_G):
                                j = g * EXP_G + jj
                                for i in range(2):
                                    nc.tensor.matmul(
                                        ctxp[i][:],
                                        vv[:, j, heads[i], :],
                                        exps[i][:, j, :],
                                        start=(j == 0), stop=(j == ST - 1),
                                    )
                        # finalize: evacuate both ctx banks first, then
                        # transpose + normalize (pso reuses the ctx slots)
                        ctx_sbs = []
                        for i in range(2):
                            ctx_sb = ctxs.tile([DH + 1, 512], F32, name="ctx_sb")
                            nc.vector.tensor_copy(ctx_sb[:], ctxp[i][:])
                            ctx_sbs.append(ctx_sb)
                        for i in range(2):
                            h = heads[i]
                            ctx_sb = ctx_sbs[i]
                            for t in range(4):
                                pso = ps_ctx.tile([128, DH + 1], F32, name="ctx")
                                nc.tensor.transpose(
                                    pso[:],
                                    ctx_sb[:, ts(t, 128)],
                                    ident_f[0:DH + 1, 0:DH + 1],
                                )
                                rec = ctxs.tile([128, 1], F32, name="rec")
                                nc.vector.reciprocal(rec[:], pso[:, DH:DH + 1])
                                nc.vector.tensor_scalar_mul(
                                    out_sb[:, t, ds(i * DH, DH)],
                                    pso[:, 0:DH], rec[:],
                                )
                                if use_bv:
                                    nc.vector.tensor_add(
                                        out_sb[:, t, ds(i * DH, DH)],
                                        out_sb[:, t, ds(i * DH, DH)],
                                        bv_bc[:, ds(h * DH, DH)],
                                    )
                        for t in range(4):
                            nc.sync.dma_start(
                                out_d[ds(c * 512 + t * 128, 128),
                                      ds(hp * 128, 128)],
                                out_sb[:, t, :],
                            )

    nc.compile()
    return nc


_programs = {}


def _get_program(use_mask: bool, use_bv: bool):
    key = (use_mask, use_bv)
    if key not in _programs:
        _programs[key] = _build(*key)
    return _programs[key]


def _shard_inputs(x, mask, Wq, bq, Wk, bk, Wv, bv):
    in_maps = []
    for c in range(N_CORES):
        b, g = divmod(c, 2)
        cols = slice(g * DHL, (g + 1) * DHL)
        in_maps.append({
            "x": np.ascontiguousarray(x[b]),
            "wq": np.ascontiguousarray(Wq[:, cols]),
            "wk": np.ascontiguousarray(Wk[:, cols]),
            "wv": np.ascontiguousarray(Wv[:, cols]),
            "bq": np.ascontiguousarray(bq[cols]),
            "bk": np.ascontiguousarray(bk[cols]),
            "bv": np.ascontiguousarray(bv[cols]),
            "mask": np.ascontiguousarray(mask[b]),
        })
    return in_maps


def _run(inputs, trace=False, **kw):
    x = np.asarray(inputs["x"], dtype=np.float32)
    mask = np.asarray(inputs["mask"], dtype=np.int32)
    use_mask = not bool(np.all(mask == 1))
    use_bv = bool(np.any(np.asarray(inputs["bv"]) != 0.0))
    nc = _get_program(use_mask, use_bv)
    in_maps = _shard_inputs(
        x, mask,
        np.asarray(inputs["Wq"], dtype=np.float32),
        np.asarray(inputs["bq"], dtype=np.float32),
        np.asarray(inputs["Wk"], dtype=np.float32),
        np.asarray(inputs["bk"], dtype=np.float32),
        np.asarray(inputs["Wv"], dtype=np.float32),
        np.asarray(inputs["bv"], dtype=np.float32),
    )
    res = run_bass_kernel_spmd(nc, in_maps, list(range(N_CORES)), trace=trace, **kw)
    out = np.empty((B, S, H * DH), np.float32)
    for c in range(N_CORES):
        b, g = divmod(c, 2)
        out[b, :, g * DHL:(g + 1) * DHL] = res.results[c]["out"]
    return out, res


def kernel(**inputs) -> np.ndarray:
    out, _ = _run(inputs)
    return out


# revision 28
# speedup vs baseline: 1.0330x; 1.0330x over previous
"""Multi-head attention layer on 8 Trainium2 NeuronCores.

Sharding: data-parallel over the batch (4) x tensor-parallel over head
groups (2 groups of 8 heads) -> 8 independent shards, one per core. Scores
and softmax stay device-local per head; no collectives are needed. Each
core computes, for its (batch, 8-head) shard:

    q/k projections in transposed layout  qT/kT = W.T @ x.T   [512, 2048]
    v projection in natural layout        v = x @ Wv          [2048, 512]
    per head: sT = k qT (scores transposed, [sk, sq]); e = exp(sT/8 + mask)
    ctxT' = [v | 1].T @ e  -> rows 0..63 = unnormalized ctx.T, row 64 = sum(e)
    transpose ctxT' back, divide by the sum row -> ctx [2048, 512]

Computing scores transposed makes the softmax reduction the contraction
dim of the PV matmul (ones-column trick), so no [S,S] transposes are ever
needed. Matmul operands are bf16 (cast for free by SWDGE DMA), accumulation
is fp32 in PSUM; exp runs on the scalar engine reading PSUM directly.
"""

import sys

if "/opt/trn_rl_repo" not in sys.path:
    sys.path.insert(0, "/opt/trn_rl_repo")

import numpy as np

import concourse.bass as bass
import concourse.mybir as mybir
import concourse.tile as tile
from concourse import bacc
from concourse.bass import ds, ts
from concourse.bass_utils import run_bass_kernel_spmd
from concourse.masks import make_identity
from concourse.tile import add_dep_helper

F32 = mybir.dt.float32
F32R = mybir.dt.float32r
BF16 = mybir.dt.bfloat16
I32 = mybir.dt.int32
EXPF = mybir.ActivationFunctionType.Exp
COPYF = mybir.ActivationFunctionType.Copy

B, S, D = 4, 2048, 1024
H, DH = 16, 64
NEG_INF = -10000.0
N_CORES = 8
HL = H // 2          # heads per core (8)
DHL = HL * DH        # local qkv width (512)
KD = D // 128        # contraction slices (8)
ST = S // 128        # sequence tiles (16)
SC = S // 512        # sequence 512-chunks (4)
SCALE = 1.0 / 8.0    # 1/sqrt(DH)
EXP_G = 2            # sk-tiles per exp instruction (2 f32 PSUM banks)


def _build(use_mask: bool, use_bv: bool):
    nc = bacc.Bacc("TRN2", target_bir_lowering=False, debug=True)

    x_d = nc.dram_tensor("x", [S, D], F32, kind="ExternalInput")
    wq_d = nc.dram_tensor("wq", [D, DHL], F32, kind="ExternalInput")
    wk_d = nc.dram_tensor("wk", [D, DHL], F32, kind="ExternalInput")
    wv_d = nc.dram_tensor("wv", [D, DHL], F32, kind="ExternalInput")
    bq_d = nc.dram_tensor("bq", [DHL], F32, kind="ExternalInput")
    bk_d = nc.dram_tensor("bk", [DHL], F32, kind="ExternalInput")
    bv_d = nc.dram_tensor("bv", [DHL], F32, kind="ExternalInput")
    mask_d = nc.dram_tensor("mask", [S], I32, kind="ExternalInput")
    out_d = nc.dram_tensor("out", [S, DHL], F32, kind="ExternalOutput")

    with tile.TileContext(nc) as tc:
        with (
            tc.tile_pool(name="persist", bufs=1) as persist,
            tc.tile_pool(name="xload", bufs=3) as xload,
            tc.tile_pool(name="qkv", bufs=1) as qkv,
        ):
            # touch Exp first so the ACT table set (which also covers the
            # Copy casts below) loads during the intro DMA window instead of
            # stalling the first real exp in the attention pipeline
            warm = persist.tile([128, 1], F32, name="warm")
            nc.gpsimd.memset(warm[:], 0.0)
            nc.scalar.activation(warm[:], warm[:], EXPF)

            ident_bf = persist.tile([128, 128], BF16, name="ident_bf")
            make_identity(nc, ident_bf[:])
            ident_f = persist.tile([128, 128], F32, name="ident_f")
            make_identity(nc, ident_f[:])

            # biases as [128, 4] columns (partition = dh within 128-block);
            # DMAs issued after the x loads (same HWDGE queue, x is critical)
            bqc = persist.tile([128, 4], F32, name="bqc")
            bkc = persist.tile([128, 4], F32, name="bkc")

            if use_mask:
                mski = persist.tile([128, ST], I32, name="mski")
                nc.sync.dma_start(mski[:], mask_d.rearrange("(t p) -> p t", p=128))
                adder = persist.tile([128, ST], F32, name="adder")
                # (1 - m) * NEG_INF == m * (-NEG_INF) + NEG_INF
                nc.scalar.activation(adder[:], mski[:], COPYF, bias=-NEG_INF * 1.0,
                                     scale=float(NEG_INF) * -1.0)
                # note: Copy computes in*scale + bias with float bias
                # in*(-NEG_INF) + NEG_INF: m=1 -> 0, m=0 -> NEG_INF

            # weights, cast to bf16 by SWDGE during the DMA (issued after the
            # x loads below -- x is on the critical path, W is not needed
            # until the first q/k projection ~45us in)
            w_all = persist.tile([128, 3, KD, DHL], BF16, name="w_all")

            if use_bv:
                bv_bf = persist.tile([1, DHL], BF16, name="bv_bf")
                nc.gpsimd.dma_start(bv_bf[:], bv_d[None, :])
                ones_col = persist.tile([1, 128], BF16, name="ones_col")
                nc.gpsimd.memset(ones_col[:], 1.0)

            qT = qkv.tile([128, 4, S], BF16, name="qT")
            kT = qkv.tile([128, 4, S], BF16, name="kT")
            vv = qkv.tile([128, ST, HL, DH + 1], BF16, name="vv")
            nc.gpsimd.memset(vv[:, :, :, DH:DH + 1], 1.0)

            # ---------------- phases: x.T -> qkv(jit) -> attention ------
            with (
                tc.tile_pool(name="xt", bufs=1) as xtp,
                tc.tile_pool(name="expp", bufs=2) as expp,
                tc.tile_pool(name="outp", bufs=2) as outp,
                tc.tile_pool(name="ctxs", bufs=4) as ctxs,
                tc.tile_pool(name="psA", bufs=2, space="PSUM") as ps_qk,
                tc.tile_pool(name="pssc", bufs=2, space="PSUM") as ps_sc,
                tc.tile_pool(name="psctx", bufs=2, space="PSUM") as ps_ctx,
            ):
                xT = xtp.tile([128, KD, S], BF16, name="xT")
                for st in range(ST):
                    x_f = xload.tile([128, D], F32, name="x_f")
                    x_dma = nc.sync.dma_start(x_f[:], x_d[ts(st, 128), :])
                    x_bf = xload.tile([128, D], BF16, name="x_bf")
                    nc.scalar.activation(x_bf[:], x_f[:], COPYF)
                    if st == 3:
                        # weight DMAs gated behind the first x tiles: both
                        # queues share the SDMA engines/HBM, and x is the
                        # critical path (W isn't consumed until ~15us in)
                        for i, wd in enumerate((wq_d, wk_d, wv_d)):
                            w_dma = nc.gpsimd.dma_start(
                                w_all[:, i, :, :],
                                wd.rearrange("(k p) m -> p k m", p=128),
                            )
                            add_dep_helper(
                                w_dma.ins, x_dma.ins,
                                reason="W cast-DMA after first x tiles",
                            )
                    for half in range(2):
                        ps_t = ps_ctx.tile([128, 4, 128], BF16, name="ctx")
                        for j in range(4):
                            dt_ = half * 4 + j
                            nc.tensor.transpose(
                                ps_t[:, j, :], x_bf[:, ts(dt_, 128)], ident_bf[:]
                            )
                        nc.vector.tensor_copy(
                            xT[:, ds(half * 4, 4), ts(st, 128)], ps_t[:]
                        )

                nc.sync.dma_start(bqc[:], bq_d.rearrange("(m p) -> p m", p=128))
                nc.sync.dma_start(bkc[:], bk_d.rearrange("(m p) -> p m", p=128))

                # q.T / k.T for one head-pair block; n4 in pairs so only two
                # PSUM banks are needed (weights reused across the pair)
                def emit_qk_gen(mb):
                    for proj, (dst, bias_c) in enumerate(((qT, bqc), (kT, bkc))):
                        for pair in range(2):
                            pss = [
                                ps_qk.tile([128, 512], F32, name="qk")
                                for _ in range(2)
                            ]
                            for kd in range(KD):
                                lhs = w_all[:, proj, kd, ts(mb, 128)]
                                for u in range(2):
                                    n4 = pair * 2 + u
                                    nc.tensor.matmul(
                                        pss[u][:], lhs, xT[:, kd, ts(n4, 512)],
                                        start=(kd == 0), stop=(kd == KD - 1),
                                    )
                                yield
                            for u in range(2):
                                nc.vector.tensor_scalar_add(
                                    dst[:, mb, ts(pair * 2 + u, 512)], pss[u][:],
                                    bias_c[:, mb:mb + 1],
                                )
                            yield

                def emit_qk(mb):
                    for _ in emit_qk_gen(mb):
                        pass

                emit_qk(0)

                # v in natural layout [S, dh_local] (+1.0 ones column)
                for st in range(ST):
                    psv = ps_qk.tile([128, 512], F32, name="qk")
                    for kd in range(KD):
                        nc.tensor.matmul(
                            psv[:], xT[:, kd, ts(st, 128)], w_all[:, 2, kd, :],
                            start=(kd == 0), stop=(kd == KD - 1),
                        )
                    nc.vector.tensor_copy(vv[:, st, :, 0:DH], psv[:])

                if use_bv:
                    bvp = ps_qk.tile([128, DHL], F32, name="qk")
                    nc.tensor.matmul(bvp[:], ones_col[:], bv_bf[:],
                                     start=True, stop=True)
                    bv_bc = persist.tile([128, DHL], F32, name="bv_bc")
                    nc.vector.tensor_copy(bv_bc[:], bvp[:])

                # ---------------- attention, head-pair outer ---------------
                NG = ST // EXP_G
                for hp in range(4):
                    if hp > 0:
                        emit_qk(hp)
                    heads = (2 * hp, 2 * hp + 1)
                    rows = (slice(0, 64), slice(64, 128))
                    for c in range(SC):
                        out_sb = outp.tile([128, 4, 128], F32, name="out_sb")
                        exps = [
                            expp.tile([128, ST, 512], BF16, name=f"exp{i}")
                            for i in range(2)
                        ]
                        ctxp = [
                            ps_ctx.tile([DH + 1, 512], F32, name="ctx")
                            for i in range(2)
                        ]

                        def emit_scores(g):
                            pair = []
                            for i in range(2):
                                t = ps_sc.tile([128, EXP_G, 512], F32, name="sc")
                                pair.append(t)
                                for jj in range(EXP_G):
                                    j = g * EXP_G + jj
                                    nc.tensor.matmul(
                                        t[:, jj, :],
                                        kT[rows[i], hp, ts(j, 128)],
                                        qT[rows[i], hp, ts(c, 512)],
                                        start=True, stop=True,
                                        tile_position=(64 * i, 0),
                                    )
                            return pair

                        pending = emit_scores(0)
                        for g in range(NG):
                            cur = pending
                            pending = emit_scores(g + 1) if g + 1 < NG else None
                            for i in range(2):
                                if use_mask:
                                    for jj in range(EXP_G):
                                        j = g * EXP_G + jj
                                        nc.scalar.activation(
                                            exps[i][:, j, :], cur[i][:, jj, :],
                                            EXPF, bias=adder[:, j:j + 1],
                                            scale=SCALE,
                                        )
                                else:
                                    nc.scalar.activation(
                                        exps[i][:, ds(g * EXP_G, EXP_G), :],
                                        cur[i][:], EXPF, scale=SCALE,
                                    )
                            for jj in range(EXP_G):
                                j = g * EXP_G + jj
                                for i in range(2):
                                    nc.tensor.matmul(
                                        ctxp[i][:],
                                        vv[:, j, heads[i], :],
                                        exps[i][:, j, :],
                                        start=(j == 0), stop=(j == ST - 1),
                                    )
                        # finalize: evacuate both ctx banks first, then
                        # transpose + normalize (pso reuses the ctx slots)
                        ctx_sbs = []
                        for i in range(2):
                            ctx_sb = ctxs.tile([DH + 1, 512], F32, name="ctx_sb")
                            nc.vector.tensor_copy(ctx_sb[:], ctxp[i][:])
                            ctx_sbs.append(ctx_sb)
                        for i in range(2):
                            h = heads[i]
                            ctx_sb = ctx_sbs[i]
                            for t in range(4):
                                pso = ps_ctx.tile([128, DH + 1], F32, name="ctx")
                                nc.tensor.transpose(
                                    pso[:],
                                    ctx_sb[:, ts(t, 128)],
                                    ident_f[0:DH + 1, 0:DH + 1],
                                )
                                rec = ctxs.tile([128, 1], F32, name="rec")
                                nc.vector.reciprocal(rec[:], pso[:, DH:DH + 1])
                                nc.vector.tensor_scalar_mul(
                                    out_sb[:, t, ds(i * DH, DH)],
                                    pso[:, 0:DH], rec[:],
                                )
                                if use_bv:
                                    nc.vector.tensor_add(
                                        out_sb[:, t, ds(i * DH, DH)],
                                        out_sb[:, t, ds(i * DH, DH)],
                                        bv_bc[:, ds(h * DH, DH)],
                                    )
                        for t in range(4):
                            nc.sync.dma_start(
                                out_d[ds(c * 512 + t * 128, 128),
                                      ds(hp * 128, 128)],
                                out_sb[:, t, :],
                            )

    nc.compile()
    return nc


_programs = {}


def _get_program(use_mask: bool, use_bv: bool):
    key = (use_mask, use_bv)
    if key not in _programs:
        _programs[key] = _build(*key)
    return _programs[key]


def _shard_inputs(x, mask, Wq, bq, Wk, bk, Wv, bv):
    in_maps = []
    for c in range(N_CORES):
        b, g = divmod(c, 2)
        cols = slice(g * DHL, (g + 1) * DHL)
        in_maps.append({
            "x": np.ascontiguousarray(x[b]),
            "wq": np.ascontiguousarray(Wq[:, cols]),
            "wk": np.ascontiguousarray(Wk[:, cols]),
            "wv": np.ascontiguousarray(Wv[:, cols]),
            "bq": np.ascontiguousarray(bq[cols]),
            "bk": np.ascontiguousarray(bk[cols]),
            "bv": np.ascontiguousarray(bv[cols]),
            "mask": np.ascontiguousarray(mask[b]),
        })
    return in_maps


def _run(inputs, trace=False, **kw):
    x = np.asarray(inputs["x"], dtype=np.float32)
    mask = np.asarray(inputs["mask"], dtype=np.int32)
    use_mask = not bool(np.all(mask == 1))
    use_bv = bool(np.any(np.asarray(inputs["bv"]) != 0.0))
    nc = _get_program(use_mask, use_bv)
    in_maps = _shard_inputs(
        x, mask,
        np.asarray(inputs["Wq"], dtype=np.float32),
        np.asarray(inputs["bq"], dtype=np.float32),
        np.asarray(inputs["Wk"], dtype=np.float32),
        np.asarray(inputs["bk"], dtype=np.float32),
        np.asarray(inputs["Wv"], dtype=np.float32),
        np.asarray(inputs["bv"], dtype=np.float32),
    )
    res = run_bass_kernel_spmd(nc, in_maps, list(range(N_CORES)), trace=trace, **kw)
    out = np.empty((B, S, H * DH), np.float32)
    for c in range(N_CORES):
        b, g = divmod(c, 2)
        out[b, :, g * DHL:(g + 1) * DHL] = res.results[c]["out"]
    return out, res


def kernel(**inputs) -> np.ndarray:
    out, _ = _run(inputs)
    return out
